# revision 25
# baseline (speedup 1.0000x reference)
"""AreaAttention Trainium2 kernel: 8-core SPMD, (batch, head-pair) sharding.

Core c handles batch b = c//2 and heads {2*(c%2), 2*(c%2)+1}.
Per core:
  qkv 1x1-conv (+BN+SiLU) for its 384 output channels,
  full 4096x4096 attention for its 2 heads,
  proj partial over its 128 channels, pairwise ReduceScatter, BN+SiLU.

v2 optimizations over the 520us baseline:
  - Q/K stored fp8e4m3; S = K^T Q via DoubleRow fp8 matmuls (2x PE rate).
    d=64 contraction packed as [32 partitions x 2 subtiles]; channel
    order permuted host-side so SiLU output + one SBUF->SBUF DMA fold
    produces the packed layout.
  - exp(S) split between ACT (true Exp) and DVE (Schraudolph bf16 bit
    trick via tensor_scalar f32->int16) to balance the two engines.
  - BN affines computed host-side (no Sqrt table load, fewer setup ops).
  - proj contraction packed (both heads in one 128-deep matmul).
  - denominator reciprocal read straight from PSUM row 64 (no DMA trip).
  - final SiLU via tanh: silu(y') = yp*(1+tanh(yp)), yp = y'/2
    (tanh lives in the exp table set -> no table thrash).
  - out2 PSUM double-buffered; rep/proj borrow st-tag PSUM buffers.

Layouts (per core):
  x_sb   [128, 2, 4096] bf16   x[b] as [c-tile, c-in-tile, token]
  qT/kT  [64, 2, 4096]  fp8    [d-fold (h0:0-31, h1:32-63), sub, token]
  v_sb   [128, 32*193]  bf16   per key tile: [v0(64) | ones(2) | 0(63) |
                               v1(64)] so h1's PV output lands on psum
                               partitions 64:128 (den on row 0) and the
                               proj contraction packs both heads.
  St     [128, 1024]    psum   S^T tile: [key, 512 h0-queries | 512 h1-q]
  P      [128, 1024]    bf16   exp(scale*St)
  out2a  [128, 512]     psum   rows 0-63 h0 attn out^T, row 64 den0
  out2b  [128, 512]     psum   row 0 den1, rows 64-127 h1 attn out^T
"""

import numpy as np
from ml_dtypes import bfloat16

import concourse.bass as bass
import concourse.bacc as bacc
import concourse.mybir as mybir
import concourse.tile as tile

F32 = mybir.dt.float32
F32R = mybir.dt.float32r
BF16 = mybir.dt.bfloat16
FP8 = mybir.dt.float8e4
I16 = mybir.dt.int16
AF = mybir.ActivationFunctionType
ALU = mybir.AluOpType
SILU = AF.Silu
DR = mybir.MatmulPerfMode.DoubleRow

EPS = 1e-5
ATTN_SCALE = 64 ** -0.5
N_CORES = 8
S = 4096           # tokens (H*W)
NKT = 32           # key tiles of 128
NQB = 8            # query blocks of 512
QB = 512
PAIRS = [[0, 1], [2, 3], [4, 5], [6, 7]]

# Schraudolph exp in bf16: bitcast(int16(round(A*x + B))) ~= exp(x)
SCH_A = (128.0 / np.log(2.0)) * ATTN_SCALE
SCH_B = 16250.7
# kt tiles whose exp runs on DVE (Schraudolph); rest on ACT (true exp)
N_SCH = 13
SCH_SET = frozenset(int(round(i * NKT / N_SCH)) for i in range(N_SCH))

# channel permutation for q/k: [h0 d0:32 | h1 d0:32 | h0 d32:64 | h1 d32:64]
QK_PERM = np.concatenate([np.arange(0, 32), np.arange(64, 96),
                          np.arange(32, 64), np.arange(96, 128)])


DEBUG = False


def build_graph():
    nc = bacc.Bacc("TRN2", target_bir_lowering=False, debug=False,
                   num_devices=N_CORES)

    x_ext = nc.dram_tensor("x", [2, 128, S], BF16, kind="ExternalInput")
    wq_ext = nc.dram_tensor("wq", [2, 128, 128], BF16, kind="ExternalInput")
    wk_ext = nc.dram_tensor("wk", [2, 128, 128], BF16, kind="ExternalInput")
    wv_ext = nc.dram_tensor("wv", [2, 128, 128], BF16, kind="ExternalInput")
    pw_ext = nc.dram_tensor("pw", [128, 256], BF16, kind="ExternalInput")
    affq_ext = nc.dram_tensor("affq", [128, 2], F32, kind="ExternalInput")
    affk_ext = nc.dram_tensor("affk", [128, 2], F32, kind="ExternalInput")
    affp_ext = nc.dram_tensor("affp", [128, 2], F32, kind="ExternalInput")
    vshift_ext = nc.dram_tensor("vshift", [1, 512], BF16,
                                kind="ExternalInput")
    out_ext = nc.dram_tensor("out", [128, S], F32, kind="ExternalOutput")

    partial_dram = nc.dram_tensor("partial", [NQB, 2, 128, QB], F32)
    red_dram = nc.dram_tensor("red", [NQB, 128, QB], F32)

    dbg = {}
    if DEBUG:
        dbg["qT"] = nc.dram_tensor("d_qT", [64, 2, S], FP8,
                                   kind="ExternalOutput")
        dbg["kT"] = nc.dram_tensor("d_kT", [64, 2, S], FP8,
                                   kind="ExternalOutput")
        dbg["v"] = nc.dram_tensor("d_v", [128, 32 * 193], BF16,
                                  kind="ExternalOutput")
        dbg["p0"] = nc.dram_tensor("d_p0", [128, 1024], BF16,
                                   kind="ExternalOutput")
        dbg["p1"] = nc.dram_tensor("d_p1", [128, 1024], BF16,
                                   kind="ExternalOutput")
        dbg["ua"] = nc.dram_tensor("d_ua", [64, QB], BF16,
                                   kind="ExternalOutput")
        dbg["ub"] = nc.dram_tensor("d_ub", [128, QB], BF16,
                                   kind="ExternalOutput")
        dbg["rp"] = nc.dram_tensor("d_rp", [128, 1024], F32,
                                   kind="ExternalOutput")
        dbg["prhs"] = nc.dram_tensor("d_prhs", [128, QB], BF16,
                                     kind="ExternalOutput")
        dbg["pj"] = nc.dram_tensor("d_pj", [2, 128, QB], F32,
                                   kind="ExternalOutput")
        dbg["y"] = nc.dram_tensor("d_y", [128, QB], F32,
                                  kind="ExternalOutput")

    with tile.TileContext(nc) as tc:
        with (
            tc.tile_pool(name="const", bufs=1) as const,
            tc.tile_pool(name="sb", bufs=1) as sb,
        ):
            # ---- persistent SBUF ----
            x_sb = const.tile([128, 2, S], BF16, name="x_sb")
            qT = const.tile([64, 2, S], FP8, name="qT")
            kT = const.tile([64, 2, S], FP8, name="kT")
            v_sb = const.tile([128, 32 * 193], BF16, name="v_sb")
            vzero = const.tile([128, 512], BF16, name="vzero")
            wq_sb = const.tile([128, 2, 128], BF16, name="wq_sb")
            wk_sb = const.tile([128, 2, 128], BF16, name="wk_sb")
            wv_sb = const.tile([128, 2, 128], BF16, name="wv_sb")
            pw_sb = const.tile([128, 256], BF16, name="pw_sb")
            affq_sb = const.tile([128, 2], F32, name="affq_sb")
            affk_sb = const.tile([128, 2], F32, name="affk_sb")
            affp_sb = const.tile([128, 2], F32, name="affp_sb")
            ones_bf = const.tile([128, 128], BF16, name="ones_bf")

            # ---- input DMAs ----
            for t in range(2):
                nc.sync.dma_start(out=wq_sb[:, t, :], in_=wq_ext[t])
                nc.sync.dma_start(out=wk_sb[:, t, :], in_=wk_ext[t])
                nc.sync.dma_start(out=wv_sb[:, t, :], in_=wv_ext[t])
            nc.sync.dma_start(out=pw_sb[:], in_=pw_ext[:])
            nc.sync.dma_start(out=affq_sb[:], in_=affq_ext[:])
            nc.sync.dma_start(out=affk_sb[:], in_=affk_ext[:])
            nc.sync.dma_start(out=affp_sb[:], in_=affp_ext[:])
            for s in range(NQB):
                blk = slice(s * QB, (s + 1) * QB)
                for t in range(2):
                    nc.sync.dma_start(out=x_sb[:, t, blk], in_=x_ext[t, :, blk])

            nc.vector.memset(vzero[:], 0.0)
            nc.sync.dma_start(out=vzero[0:1, :], in_=vshift_ext[:])
            # per 193-col key block: ones at cols 64,65; zeros at 66..128
            _vb = v_sb[:, 64:66]
            v_ones_ap = bass.AP(tensor=_vb.tensor, offset=_vb.offset,
                                ap=[_vb.ap[0], [193, 32], [1, 2]])
            nc.vector.memset(v_ones_ap, 1.0)
            _vz = v_sb[:, 66:129]
            v_zero_ap = bass.AP(tensor=_vz.tensor, offset=_vz.offset,
                                ap=[_vz.ap[0], [193, 32], [1, 63]])
            nc.vector.memset(v_zero_ap, 0.0)
            nc.vector.memset(ones_bf[:], 1.0)

            ps = tc.alloc_tile_pool(name="ps_pre", bufs=1, space="PSUM")

            # ---- qkv projection + BN + SiLU ----
            def qk_block(s, w_sb, aff, dstT, tag):
                blk = slice(s * QB, (s + 1) * QB)
                pp = ps.tile([128, QB], F32, name=f"{tag}p{s}", tag="qk",
                             bufs=2)
                nc.tensor.matmul(pp[:], w_sb[:, 0, :], x_sb[:, 0, blk],
                                 start=True, stop=False)
                nc.tensor.matmul(pp[:], w_sb[:, 1, :], x_sb[:, 1, blk],
                                 start=False, stop=True)
                f8 = sb.tile([128, QB], FP8, name=f"{tag}8_{s}", tag="qk8",
                             bufs=3)
                nc.scalar.activation(f8[:], pp[:], SILU,
                                     bias=aff[:, 1:2], scale=aff[:, 0:1])
                # fold [128, 512] -> [64, 2, 512]: sub t from partitions 64t+
                nc.sync.dma_start(out=dstT[0:64, 0, blk], in_=f8[0:64, :])
                nc.sync.dma_start(out=dstT[0:64, 1, blk], in_=f8[64:128, :])

            def v_block(s):
                vp = ps.tile([128, QB], F32, name=f"vp{s}", tag="vp", bufs=2)
                nc.tensor.matmul(vp[:], ones_bf[:], vzero[:],
                                 start=True, stop=False,
                                 skip_group_check=True)
                for st in range(4):
                    tok = s * 4 + st
                    col = slice(st * 128, st * 128 + 128)
                    tk = slice(tok * 128, tok * 128 + 128)
                    nc.tensor.matmul(vp[:, col], x_sb[:, 0, tk], wv_sb[:, 0, :],
                                     start=False, stop=False,
                                     skip_group_check=True)
                    nc.tensor.matmul(vp[:, col], x_sb[:, 1, tk], wv_sb[:, 1, :],
                                     start=False, stop=(st == 3),
                                     skip_group_check=True)
                # h0 slice at +0, h1 slice at +129 within each 193-col block
                _vo = v_sb[:, s * 772: s * 772 + 772]
                v_out_ap = bass.AP(tensor=_vo.tensor, offset=_vo.offset,
                                   ap=[_vo.ap[0], [193, 4], [129, 2], [1, 64]])
                vp_view = vp[:].rearrange("p (g h c) -> p g h c", g=4, h=2)
                nc.scalar.activation(v_out_ap, vp_view, SILU)

            for s_i in range(NQB):
                qk_block(s_i, wk_sb, affk_sb, kT, "k")
            for s_i in range(NQB):
                qk_block(s_i, wq_sb, affq_sb, qT, "q")
            for s_i in range(NQB):
                v_block(s_i)

            if DEBUG:
                nc.sync.dma_start(out=dbg["qT"][:], in_=qT[:])
                nc.sync.dma_start(out=dbg["kT"][:], in_=kT[:])
                nc.sync.dma_start(out=dbg["v"][:], in_=v_sb[:])

            ps.release()
            ps = tc.alloc_tile_pool(name="ps_att", bufs=1, space="PSUM")

            # ---- attention ----
            def final_stage(fq):
                # silu(y') = yp*(1+tanh(yp)), yp = (sp*y+hp)/2; affp = sp/2,hp/2
                fblk = slice(fq * QB, (fq + 1) * QB)
                y_sb = sb.tile([128, QB], F32, name=f"y_{fq}", tag="y", bufs=2)
                nc.sync.dma_start(out=y_sb[:], in_=red_dram[fq])
                if DEBUG and fq == 0:
                    nc.sync.dma_start(out=dbg["y"][:], in_=y_sb[:])
                th = sb.tile([128, QB], F32, name=f"th_{fq}", tag="th", bufs=2)
                nc.scalar.activation(th[:], y_sb[:], AF.Tanh,
                                     bias=affp_sb[:, 1:2],
                                     scale=affp_sb[:, 0:1])
                yp = sb.tile([128, QB], F32, name=f"yp_{fq}", tag="yp", bufs=2)
                nc.vector.tensor_scalar(yp[:], y_sb[:], affp_sb[:, 0:1],
                                        affp_sb[:, 1:2], ALU.mult, ALU.add)
                yo = sb.tile([128, QB], F32, name=f"yo_{fq}", tag="yo", bufs=2)
                nc.vector.scalar_tensor_tensor(yo[:], th[:], 1.0, yp[:],
                                               ALU.add, ALU.mult)
                nc.sync.dma_start(out=out_ext[:, fblk], in_=yo[:])

            for qb in range(NQB):
                qblk = slice(qb * QB, (qb + 1) * QB)
                # out2a/out2b rows 0-63: per-head attn out; row 64: den
                out2a = ps.tile([128, QB], F32, name=f"out2a_{qb}",
                                tag="out2a", bufs=2)
                out2b = ps.tile([128, QB], F32, name=f"out2b_{qb}",
                                tag="out2b", bufs=2)
                p_tiles = [None] * NKT
                for kt in range(NKT + 2):
                    if kt < NKT:
                        kblk = slice(kt * 128, (kt + 1) * 128)
                        st_t = ps.tile([128, 1024], F32, name=f"st_{qb}_{kt}",
                                       tag="st", bufs=2)
                        nc.tensor.matmul(st_t[:, 0:QB], kT[0:32, :, kblk],
                                         qT[0:32, :, qblk], start=True,
                                         stop=True, perf_mode=DR)
                        nc.tensor.matmul(st_t[:, QB:1024], kT[32:64, :, kblk],
                                         qT[32:64, :, qblk], start=True,
                                         stop=True, perf_mode=DR)
                    if kt >= 2:
                        pk = kt - 2
                        pt = p_tiles[pk]
                        first = pk == 0
                        last = pk == NKT - 1
                        vc = pk * 193
                        nc.tensor.matmul(out2a[0:65, :], v_sb[:, vc:vc + 65],
                                         pt[:, 0:QB], start=first, stop=last)
                        nc.tensor.matmul(out2b[:],
                                         v_sb[:, vc + 65:vc + 193],
                                         pt[:, QB:1024], start=first,
                                         stop=last)
                    if kt < NKT:
                        p_t = sb.tile([128, 1024], BF16, name=f"p_{qb}_{kt}",
                                      tag="p", bufs=4)
                        if kt in SCH_SET:
                            nc.vector.tensor_scalar(
                                p_t[:].bitcast(I16), st_t[:],
                                float(SCH_A), float(SCH_B), ALU.mult, ALU.add)
                        else:
                            nc.scalar.activation(p_t[:], st_t[:], AF.Exp,
                                                 scale=ATTN_SCALE)
                        p_tiles[kt] = p_t
                        if DEBUG and qb == 0 and kt < 2:
                            nc.sync.dma_start(out=dbg[f"p{kt}"][:],
                                              in_=p_t[:])

                # tail: normalize + proj partial + ReduceScatter
                # ua row 64 = den0; ub row 0 = den1, rows 64:127 h1 out
                ua = sb.tile([65, QB], BF16, name=f"ua_{qb}", tag="ua", bufs=2)
                ub = sb.tile([128, QB], BF16, name=f"ub_{qb}", tag="ub",
                             bufs=2)
                nc.vector.tensor_copy(ua[:], out2a[0:65, :])
                nc.vector.tensor_copy(ub[:], out2b[:])
                # broadcast raw dens to all partitions (K=1 matmuls), then
                # one full-tile reciprocal at partition base 0
                rpt = ps.tile([128, 1024], F32, name=f"rp_{qb}", tag="st",
                              bufs=2)
                nc.tensor.matmul(rpt[:, 0:QB], ones_bf[64:65, :],
                                 ua[64:65, :], start=True, stop=True)
                nc.tensor.matmul(rpt[:, QB:1024], ones_bf[0:1, :],
                                 ub[0:1, :], start=True, stop=True)
                rrep = sb.tile([128, 1024], F32, name=f"rr_{qb}", tag="rrep",
                               bufs=2)
                nc.vector.reciprocal_approx_fast(out=rrep[:], in_=rpt[:])
                prhs = sb.tile([128, QB], BF16, name=f"prhs_{qb}", tag="prhs",
                               bufs=2)
                nc.vector.tensor_mul(prhs[0:64, :], ua[0:64, :],
                                     rrep[0:64, 0:QB])
                nc.vector.tensor_mul(prhs[64:128, :], ub[64:128, :],
                                     rrep[64:128, QB:1024])
                if DEBUG and qb == 0:
                    nc.sync.dma_start(out=dbg["ua"][:], in_=ua[0:64, :])
                    nc.sync.dma_start(out=dbg["ub"][:], in_=ub[:])
                    nc.sync.dma_start(out=dbg["rp"][:], in_=rrep[:])
                    nc.sync.dma_start(out=dbg["prhs"][:], in_=prhs[:])
                projt = ps.tile([128, 1024], F32, name=f"pj_{qb}", tag="st",
                                bufs=2)
                for j in range(2):
                    jc = slice(j * QB, (j + 1) * QB)
                    nc.tensor.matmul(projt[:, jc],
                                     pw_sb[:, j * 128:(j + 1) * 128],
                                     prhs[:], start=True, stop=True)
                    projsb = sb.tile([128, QB], F32, name=f"pjs_{qb}_{j}",
                                     tag="projsb", bufs=4)
                    nc.vector.tensor_copy(projsb[:], projt[:, jc])
                    nc.sync.dma_start(out=partial_dram[qb, j], in_=projsb[:])
                    if DEBUG and qb == 0:
                        nc.sync.dma_start(out=dbg["pj"][j], in_=projsb[:])
                nc.gpsimd.collective_compute(
                    "ReduceScatter", ALU.add,
                    replica_groups=PAIRS,
                    ins=[partial_dram[qb]],
                    outs=[red_dram[qb]],
                )

                # final stage for qb-4 (gives the ReduceScatter 4 iterations
                # of slack so the in-order ACT engine never stalls on it)
                if qb >= 4:
                    final_stage(qb - 4)

            for fq in range(NQB - 4, NQB):
                final_stage(fq)
            ps.release()

    nc.compile()
    return nc


_NC = None


def _get_nc():
    global _NC
    if _NC is None:
        _NC = build_graph()
    return _NC


def _prep_core_inputs(inputs, c):
    b, hp = c // 2, c % 2
    x = np.asarray(inputs["x"][b], np.float32).reshape(256, S)
    xbf = np.ascontiguousarray(x.astype(bfloat16).reshape(2, 128, S))
    W = np.asarray(inputs["qkv_w"], np.float32)

    g = np.asarray(inputs["qkv_gamma"], np.float32)
    be = np.asarray(inputs["qkv_beta"], np.float32)
    m = np.asarray(inputs["qkv_mean"], np.float32)
    v = np.asarray(inputs["qkv_var"], np.float32)

    def bn_aff(lo):
        sc = (g[lo:lo + 128] / np.sqrt(v[lo:lo + 128] + EPS))
        sh = be[lo:lo + 128] - m[lo:lo + 128] * sc
        return sc, sh

    def wslice(sec, perm=None):
        rows = W[sec * 256 + 128 * hp: sec * 256 + 128 * hp + 128, :]
        wt = rows.T  # [256 in, 128 out]
        if perm is not None:
            wt = wt[:, perm]
        return np.ascontiguousarray(
            wt.astype(bfloat16).reshape(2, 128, 128))

    qlo, klo, vlo = 128 * hp, 256 + 128 * hp, 512 + 128 * hp
    wq = wslice(0, QK_PERM)
    wk = wslice(1, QK_PERM)
    sc_v, sh_v = bn_aff(vlo)
    rows_v = W[vlo:vlo + 128, :].T * sc_v[None, :]  # fold BN scale into wv
    wv = np.ascontiguousarray(rows_v.astype(bfloat16).reshape(2, 128, 128))
    vshift = np.ascontiguousarray(np.tile(sh_v, 4)[None, :].astype(bfloat16))

    sc_q, sh_q = bn_aff(qlo)
    affq = np.stack([sc_q[QK_PERM], sh_q[QK_PERM]], axis=1)
    sc_k, sh_k = bn_aff(klo)
    affk = np.stack([sc_k[QK_PERM], sh_k[QK_PERM]], axis=1)

    pwf = np.asarray(inputs["proj_w"], np.float32)
    pw = np.ascontiguousarray(
        pwf[:, 128 * hp: 128 * hp + 128].T.astype(bfloat16))  # [128in, 256out]

    pg = np.asarray(inputs["proj_gamma"], np.float32)
    pb = np.asarray(inputs["proj_beta"], np.float32)
    pm = np.asarray(inputs["proj_mean"], np.float32)
    pv = np.asarray(inputs["proj_var"], np.float32)
    plo = 128 * hp
    sc_p = pg[plo:plo + 128] / np.sqrt(pv[plo:plo + 128] + EPS)
    sh_p = pb[plo:plo + 128] - pm[plo:plo + 128] * sc_p
    affp = np.stack([sc_p * 0.5, sh_p * 0.5], axis=1)

    return {
        "x": xbf, "wq": wq, "wk": wk, "wv": wv, "pw": pw,
        "affq": np.ascontiguousarray(affq, np.float32),
        "affk": np.ascontiguousarray(affk, np.float32),
        "affp": np.ascontiguousarray(affp, np.float32),
        "vshift": vshift,
    }


def run(inputs, trace=False):
    from concourse.bass_utils import run_bass_kernel_spmd
    nc = _get_nc()
    in_maps = [_prep_core_inputs(inputs, c) for c in range(N_CORES)]
    res = run_bass_kernel_spmd(nc, in_maps, list(range(N_CORES)), trace=trace)
    out = np.empty((4, 256, S), np.float32)
    for c in range(N_CORES):
        b, hp = c // 2, c % 2
        out[b, 128 * hp: 128 * hp + 128, :] = res.results[c]["out"]
    return out.reshape(4, 256, 64, 64), res


def kernel(**inputs):
    out, _ = run(inputs)
    return out


# revision 29
# speedup vs baseline: 1.7663x; 1.7663x over previous
"""AreaAttention Trainium2 kernel: 8-core SPMD, (batch, head-pair) sharding.

Core c handles batch b = c//2 and heads {2*(c%2), 2*(c%2)+1}.
Per core:
  qkv 1x1-conv (+BN+SiLU) for its 384 output channels,
  full 4096x4096 attention for its 2 heads,
  proj partial over its 128 channels, pairwise ReduceScatter, BN+SiLU.

v3 design notes (hardware-measured):
  - fp8 matmul runs at 2 cyc/row on this hw (no DoubleRow speedup) ->
    everything stays bf16.
  - QK: per-head K=64 matmuls (kT/qT rows 0:64 = head0 d, 64:128 =
    head1 d; no zero padding, no extra copies -- ACT SiLU writes qT/kT
    directly).
  - PV runs TRANSPOSED: P^T tiles ([keys, queries], i.e. the exp output
    as-is) are the stationary, V[keys, 65] the moving (65-col moving
    benches at ~36ns/mm, ldweights hides).  Output out_t[q, ch] gets the
    softmax denominator in col 64 of each 65-block via the ones column,
    so normalization is a per-partition tensor_scalar (no broadcast
    matmul, no reciprocal-of-row-64 machinery).
  - exp(S) split between ACT (true Exp) and DVE (Schraudolph bf16 bit
    trick via tensor_scalar f32->int16) to balance engines.
  - BN affines computed host-side; v-BN scale folded into wv, shift
    injected via a ones x vzero matmul into the v PSUM accumulator.
  - final SiLU via tanh (same ACT table set as Exp -> no table thrash).

Layouts (per core):
  x_sb   [128, 2, 4096] bf16   x[b] as [c-tile, c-in-tile, token]
  qT/kT  [128, 4096]    bf16   [channel (h0 d: 0-63, h1 d: 64-127), tok]
  v_sb   [128, 32*130]  bf16   [key-in-tile, tile*130 + head*65 + ch],
                               col 64/129 per tile = ones (denominator)
  St     [128, 1024]    psum   S^T tile: [key, 512 h0-queries | 512 h1-q]
  P      [128, 1024]    bf16   exp(scale*St)
  out_t  [128, 520]     psum   [query, (qt,h,ch)]; den at col 64 of each
                               65-block
"""

import numpy as np
from ml_dtypes import bfloat16

import concourse.bass as bass
import concourse.bacc as bacc
import concourse.mybir as mybir
import concourse.tile as tile

F32 = mybir.dt.float32
BF16 = mybir.dt.bfloat16
I16 = mybir.dt.int16
AF = mybir.ActivationFunctionType
ALU = mybir.AluOpType
SILU = AF.Silu

EPS = 1e-5
ATTN_SCALE = 64 ** -0.5
N_CORES = 8
S = 4096           # tokens (H*W)
NKT = 32           # key tiles of 128
NQB = 8            # query blocks of 512
QB = 512
PAIRS = [[0, 1], [2, 3], [4, 5], [6, 7]]

# Schraudolph exp in bf16: bitcast(int16(round(A*x + B))) ~= exp(x)
SCH_A = (128.0 / np.log(2.0)) * ATTN_SCALE
SCH_B = 16250.7
# kt tiles whose exp runs on DVE (Schraudolph); rest on ACT (true exp)
N_SCH = 13
SCH_SET = frozenset(int(round(i * NKT / N_SCH)) for i in range(N_SCH))

DEBUG = False


def build_graph():
    nc = bacc.Bacc("TRN2", target_bir_lowering=False, debug=False,
                   num_devices=N_CORES)

    x_ext = nc.dram_tensor("x", [2, 128, S], BF16, kind="ExternalInput")
    wq_ext = nc.dram_tensor("wq", [2, 128, 128], BF16, kind="ExternalInput")
    wk_ext = nc.dram_tensor("wk", [2, 128, 128], BF16, kind="ExternalInput")
    wv_ext = nc.dram_tensor("wv", [2, 128, 128], BF16, kind="ExternalInput")
    pw_ext = nc.dram_tensor("pw", [128, 256], BF16, kind="ExternalInput")
    affq_ext = nc.dram_tensor("affq", [128, 2], F32, kind="ExternalInput")
    affk_ext = nc.dram_tensor("affk", [128, 2], F32, kind="ExternalInput")
    affp_ext = nc.dram_tensor("affp", [128, 2], F32, kind="ExternalInput")
    vshift_ext = nc.dram_tensor("vshift", [1, 512], BF16,
                                kind="ExternalInput")
    eye_ext = nc.dram_tensor("eye", [128, 128], F32, kind="ExternalInput")
    out_ext = nc.dram_tensor("out", [128, S], F32, kind="ExternalOutput")

    partial_dram = nc.dram_tensor("partial", [NQB, 2, 128, QB], F32)
    red_dram = nc.dram_tensor("red", [NQB, 128, QB], F32)

    dbg = {}
    if DEBUG:
        dbg["p0"] = nc.dram_tensor("d_p0", [128, 1024], BF16,
                                   kind="ExternalOutput")
        dbg["p1"] = nc.dram_tensor("d_p1", [128, 1024], BF16,
                                   kind="ExternalOutput")
        dbg["prhsq"] = nc.dram_tensor("d_prhsq", [128, QB], F32,
                                      kind="ExternalOutput")
        dbg["prhs"] = nc.dram_tensor("d_prhs", [128, QB], BF16,
                                     kind="ExternalOutput")
        dbg["pj"] = nc.dram_tensor("d_pj", [2, 128, QB], F32,
                                   kind="ExternalOutput")

    with tile.TileContext(nc) as tc:
        with (
            tc.tile_pool(name="const", bufs=1) as const,
            tc.tile_pool(name="sb", bufs=1) as sb,
        ):
            # ---- persistent SBUF ----
            x_sb = const.tile([128, 2, S], BF16, name="x_sb")
            qT = const.tile([128, S], BF16, name="qT")
            kT = const.tile([128, S], BF16, name="kT")
            v_sb = const.tile([128, 32 * 130], BF16, name="v_sb")
            vzero = const.tile([128, 512], BF16, name="vzero")
            wq_sb = const.tile([128, 2, 128], BF16, name="wq_sb")
            wk_sb = const.tile([128, 2, 128], BF16, name="wk_sb")
            wv_sb = const.tile([128, 2, 128], BF16, name="wv_sb")
            pw_sb = const.tile([128, 256], BF16, name="pw_sb")
            affq_sb = const.tile([128, 2], F32, name="affq_sb")
            affk_sb = const.tile([128, 2], F32, name="affk_sb")
            affp_sb = const.tile([128, 2], F32, name="affp_sb")
            eye_sb = const.tile([128, 128], F32, name="eye_sb")
            ones_bf = const.tile([128, 128], BF16, name="ones_bf")

            # ---- input DMAs ----
            for t in range(2):
                nc.sync.dma_start(out=wq_sb[:, t, :], in_=wq_ext[t])
                nc.sync.dma_start(out=wk_sb[:, t, :], in_=wk_ext[t])
                nc.sync.dma_start(out=wv_sb[:, t, :], in_=wv_ext[t])
            nc.sync.dma_start(out=pw_sb[:], in_=pw_ext[:])
            nc.sync.dma_start(out=affq_sb[:], in_=affq_ext[:])
            nc.sync.dma_start(out=affk_sb[:], in_=affk_ext[:])
            nc.sync.dma_start(out=affp_sb[:], in_=affp_ext[:])
            nc.sync.dma_start(out=eye_sb[:], in_=eye_ext[:])
            for s in range(NQB):
                blk = slice(s * QB, (s + 1) * QB)
                for t in range(2):
                    nc.sync.dma_start(out=x_sb[:, t, blk], in_=x_ext[t, :, blk])

            nc.vector.memset(vzero[:], 0.0)
            nc.sync.dma_start(out=vzero[0:1, :], in_=vshift_ext[:])
            # ones columns at 64 + 129 of each 130-col key block
            _vb = v_sb[:, 64:65]
            v_ones_ap = bass.AP(tensor=_vb.tensor, offset=_vb.offset,
                                ap=[_vb.ap[0], [130, 32], [65, 2]])
            nc.vector.memset(v_ones_ap, 1.0)
            nc.vector.memset(ones_bf[:], 1.0)

            ps = tc.alloc_tile_pool(name="ps_pre", bufs=1, space="PSUM")

            # ---- qkv projection + BN + SiLU ----
            def qk_block(s, w_sb, aff, dstT, tag):
                blk = slice(s * QB, (s + 1) * QB)
                pp = ps.tile([128, QB], F32, name=f"{tag}p{s}", tag="qk",
                             bufs=2)
                nc.tensor.matmul(pp[:], w_sb[:, 0, :], x_sb[:, 0, blk],
                                 start=True, stop=False)
                nc.tensor.matmul(pp[:], w_sb[:, 1, :], x_sb[:, 1, blk],
                                 start=False, stop=True)
                nc.scalar.activation(dstT[:, blk], pp[:], SILU,
                                     bias=aff[:, 1:2], scale=aff[:, 0:1])

            def v_block(s):
                vp = ps.tile([128, QB], F32, name=f"vp{s}", tag="vp", bufs=2)
                nc.tensor.matmul(vp[:], ones_bf[:], vzero[:],
                                 start=True, stop=False,
                                 skip_group_check=True)
                for st in range(4):
                    tok = s * 4 + st
                    col = slice(st * 128, st * 128 + 128)
                    tk = slice(tok * 128, tok * 128 + 128)
                    nc.tensor.matmul(vp[:, col], x_sb[:, 0, tk], wv_sb[:, 0, :],
                                     start=False, stop=False,
                                     skip_group_check=True)
                    nc.tensor.matmul(vp[:, col], x_sb[:, 1, tk], wv_sb[:, 1, :],
                                     start=False, stop=(st == 3),
                                     skip_group_check=True)
                _vo = v_sb[:, s * 520: s * 520 + 520]
                v_out_ap = bass.AP(tensor=_vo.tensor, offset=_vo.offset,
                                   ap=[_vo.ap[0], [130, 4], [65, 2], [1, 64]])
                vp_view = vp[:].rearrange("p (g h c) -> p g h c", g=4, h=2)
                nc.scalar.activation(v_out_ap, vp_view, SILU)

            for s_i in range(NQB):
                qk_block(s_i, wk_sb, affk_sb, kT, "k")
            for s_i in range(NQB):
                qk_block(s_i, wq_sb, affq_sb, qT, "q")
            for s_i in range(NQB):
                v_block(s_i)

            ps.release()
            ps = tc.alloc_tile_pool(name="ps_att", bufs=1, space="PSUM")

            # ---- attention ----
            def final_stage(fq):
                # silu(y') = yp*(1+tanh(yp)), yp = (sp*y+hp)/2; affp = sp/2,hp/2
                fblk = slice(fq * QB, (fq + 1) * QB)
                y_sb = sb.tile([128, QB], F32, name=f"y_{fq}", tag="y", bufs=2)
                nc.sync.dma_start(out=y_sb[:], in_=red_dram[fq])
                th = sb.tile([128, QB], F32, name=f"th_{fq}", tag="th", bufs=2)
                nc.scalar.activation(th[:], y_sb[:], AF.Tanh,
                                     bias=affp_sb[:, 1:2],
                                     scale=affp_sb[:, 0:1])
                yp = sb.tile([128, QB], F32, name=f"yp_{fq}", tag="yp", bufs=2)
                nc.vector.tensor_scalar(yp[:], y_sb[:], affp_sb[:, 0:1],
                                        affp_sb[:, 1:2], ALU.mult, ALU.add)
                yo = sb.tile([128, QB], F32, name=f"yo_{fq}", tag="yo", bufs=2)
                nc.vector.scalar_tensor_tensor(yo[:], th[:], 1.0, yp[:],
                                               ALU.add, ALU.mult)
                nc.sync.dma_start(out=out_ext[:, fblk], in_=yo[:])

            for qb in range(NQB):
                qblk = slice(qb * QB, (qb + 1) * QB)
                # out_t[q, qt*130 + h*65 + c]; den at c=64
                # start=True only resets the 2KB PSUM bank containing the
                # region start, so the 65-col accumulation regions can't
                # rely on it.  Reset both banks with two explicit
                # zero-matmuls, then accumulate with start=False.
                out_t = ps.tile([128, 1024], F32, name=f"ot_{qb}",
                                tag="out_t", bufs=2)
                for bank in range(2):
                    nc.tensor.matmul(out_t[:, bank * QB:(bank + 1) * QB],
                                     vzero[32:33, 0:128],
                                     vzero[32:33, 0:QB],
                                     start=True, stop=False,
                                     skip_group_check=True)
                p_tiles = [None] * NKT
                for kt in range(NKT + 2):
                    if kt < NKT:
                        kblk = slice(kt * 128, (kt + 1) * 128)
                        st_t = ps.tile([128, 1024], F32, name=f"st_{qb}_{kt}",
                                       tag="st", bufs=2)
                        nc.tensor.matmul(st_t[:, 0:QB], kT[0:64, kblk],
                                         qT[0:64, qblk], start=True,
                                         stop=True)
                        nc.tensor.matmul(st_t[:, QB:1024], kT[64:128, kblk],
                                         qT[64:128, qblk], start=True,
                                         stop=True)
                    if kt >= 2:
                        pk = kt - 2
                        pt = p_tiles[pk]
                        last = pk == NKT - 1
                        vc = pk * 130
                        for qt in range(4):
                            for h in range(2):
                                oc = qt * 130 + h * 65
                                nc.tensor.matmul(
                                    out_t[:, oc:oc + 65],
                                    pt[:, h * QB + qt * 128:
                                       h * QB + qt * 128 + 128],
                                    v_sb[:, vc + h * 65: vc + h * 65 + 65],
                                    start=False,
                                    stop=last and qt == 3 and h == 1,
                                    skip_group_check=True)
                    if kt < NKT:
                        p_t = sb.tile([128, 1024], BF16, name=f"p_{qb}_{kt}",
                                      tag="p", bufs=4)
                        if kt in SCH_SET:
                            nc.vector.tensor_scalar(
                                p_t[:].bitcast(I16), st_t[:],
                                float(SCH_A), float(SCH_B), ALU.mult, ALU.add)
                        else:
                            nc.scalar.activation(p_t[:], st_t[:], AF.Exp,
                                                 scale=ATTN_SCALE)
                        p_tiles[kt] = p_t
                        if DEBUG and qb == 0 and kt < 2:
                            nc.sync.dma_start(out=dbg[f"p{kt}"][:],
                                              in_=p_t[:])

                # tail: per-partition normalize, transpose, proj, RS
                dn = sb.tile([128, 8], F32, name=f"dn_{qb}", tag="dn", bufs=2)
                _ot = out_t[:, 64:65]
                den_ap = bass.AP(tensor=_ot.tensor, offset=_ot.offset,
                                 ap=[_ot.ap[0], [65, 8]])
                nc.vector.tensor_copy(dn[:], den_ap)
                rn = sb.tile([128, 8], F32, name=f"rn_{qb}", tag="rn", bufs=2)
                nc.vector.reciprocal_approx_fast(out=rn[:], in_=dn[:])
                prhsq = sb.tile([128, QB], F32, name=f"pq_{qb}", tag="prhsq",
                                bufs=2)
                for qt in range(4):
                    for h in range(2):
                        oc = qt * 130 + h * 65
                        dc = qt * 128 + h * 64
                        nc.vector.tensor_scalar_mul(
                            prhsq[:, dc:dc + 64], out_t[:, oc:oc + 64],
                            rn[:, 2 * qt + h: 2 * qt + h + 1])
                trans = ps.tile([128, 1024], F32, name=f"tr_{qb}", tag="st",
                                bufs=2)
                for qt in range(4):
                    qc = slice(qt * 128, (qt + 1) * 128)
                    nc.tensor.transpose(trans[:, qc], prhsq[:, qc], eye_sb[:])
                prhs = sb.tile([128, QB], BF16, name=f"prhs_{qb}", tag="prhs",
                               bufs=2)
                nc.vector.tensor_copy(prhs[:], trans[:, 0:QB])
                if DEBUG and qb == 0:
                    nc.sync.dma_start(out=dbg["prhsq"][:], in_=prhsq[:])
                    nc.sync.dma_start(out=dbg["prhs"][:], in_=prhs[:])
                projt = ps.tile([128, 1024], F32, name=f"pj_{qb}", tag="st",
                                bufs=2)
                for j in range(2):
                    jc = slice(j * QB, (j + 1) * QB)
                    nc.tensor.matmul(projt[:, jc],
                                     pw_sb[:, j * 128:(j + 1) * 128],
                                     prhs[:], start=True, stop=True)
                    projsb = sb.tile([128, QB], F32, name=f"pjs_{qb}_{j}",
                                     tag="projsb", bufs=4)
                    nc.vector.tensor_copy(projsb[:], projt[:, jc])
                    nc.sync.dma_start(out=partial_dram[qb, j], in_=projsb[:])
                    if DEBUG and qb == 0:
                        nc.sync.dma_start(out=dbg["pj"][j], in_=projsb[:])
                nc.gpsimd.collective_compute(
                    "ReduceScatter", ALU.add,
                    replica_groups=PAIRS,
                    ins=[partial_dram[qb]],
                    outs=[red_dram[qb]],
                )

                # final stage for qb-4 (gives the ReduceScatter 4 iterations
                # of slack so the in-order ACT engine never stalls on it)
                if qb >= 4:
                    final_stage(qb - 4)

            for fq in range(NQB - 4, NQB):
                final_stage(fq)
            ps.release()

    nc.compile()
    return nc


_NC = None


def _get_nc():
    global _NC
    if _NC is None:
        _NC = build_graph()
    return _NC


def _prep_core_inputs(inputs, c):
    b, hp = c // 2, c % 2
    x = np.asarray(inputs["x"][b], np.float32).reshape(256, S)
    xbf = np.ascontiguousarray(x.astype(bfloat16).reshape(2, 128, S))
    W = np.asarray(inputs["qkv_w"], np.float32)

    g = np.asarray(inputs["qkv_gamma"], np.float32)
    be = np.asarray(inputs["qkv_beta"], np.float32)
    m = np.asarray(inputs["qkv_mean"], np.float32)
    v = np.asarray(inputs["qkv_var"], np.float32)

    def bn_aff(lo):
        sc = (g[lo:lo + 128] / np.sqrt(v[lo:lo + 128] + EPS))
        sh = be[lo:lo + 128] - m[lo:lo + 128] * sc
        return sc, sh

    def wslice(sec):
        rows = W[sec * 256 + 128 * hp: sec * 256 + 128 * hp + 128, :]
        return np.ascontiguousarray(
            rows.T.astype(bfloat16).reshape(2, 128, 128))

    qlo, klo, vlo = 128 * hp, 256 + 128 * hp, 512 + 128 * hp
    wq = wslice(0)
    wk = wslice(1)
    sc_v, sh_v = bn_aff(vlo)
    rows_v = W[vlo:vlo + 128, :].T * sc_v[None, :]  # fold BN scale into wv
    wv = np.ascontiguousarray(rows_v.astype(bfloat16).reshape(2, 128, 128))
    vshift = np.ascontiguousarray(np.tile(sh_v, 4)[None, :].astype(bfloat16))

    sc_q, sh_q = bn_aff(qlo)
    affq = np.stack([sc_q, sh_q], axis=1)
    sc_k, sh_k = bn_aff(klo)
    affk = np.stack([sc_k, sh_k], axis=1)

    pwf = np.asarray(inputs["proj_w"], np.float32)
    pw = np.ascontiguousarray(
        pwf[:, 128 * hp: 128 * hp + 128].T.astype(bfloat16))  # [128in, 256out]

    pg = np.asarray(inputs["proj_gamma"], np.float32)
    pb = np.asarray(inputs["proj_beta"], np.float32)
    pm = np.asarray(inputs["proj_mean"], np.float32)
    pv = np.asarray(inputs["proj_var"], np.float32)
    plo = 128 * hp
    sc_p = pg[plo:plo + 128] / np.sqrt(pv[plo:plo + 128] + EPS)
    sh_p = pb[plo:plo + 128] - pm[plo:plo + 128] * sc_p
    affp = np.stack([sc_p * 0.5, sh_p * 0.5], axis=1)

    return {
        "x": xbf, "wq": wq, "wk": wk, "wv": wv, "pw": pw,
        "affq": np.ascontiguousarray(affq, np.float32),
        "affk": np.ascontiguousarray(affk, np.float32),
        "affp": np.ascontiguousarray(affp, np.float32),
        "vshift": vshift,
        "eye": np.eye(128, dtype=np.float32),
    }


def run(inputs, trace=False):
    from concourse.bass_utils import run_bass_kernel_spmd
    nc = _get_nc()
    in_maps = [_prep_core_inputs(inputs, c) for c in range(N_CORES)]
    res = run_bass_kernel_spmd(nc, in_maps, list(range(N_CORES)), trace=trace)
    out = np.empty((4, 256, S), np.float32)
    for c in range(N_CORES):
        b, hp = c // 2, c % 2
        out[b, 128 * hp: 128 * hp + 128, :] = res.results[c]["out"]
    return out.reshape(4, 256, 64, 64), res


def kernel(**inputs):
    out, _ = run(inputs)
    return out
